# revision 9
# baseline (speedup 1.0000x reference)
"""4-D average pool (kernel=2, stride=2) over [2,16,32,32,32,32] f32, on 8 NeuronCores.

Strategy: data-parallel over the 32 (b,c) slices -> 4 slices per core.
Per core the input is a contiguous [4096, 1024] f32 block (rows = (slice,d1,d2),
cols = (d3,d4)).  Per 2 MiB load tile [128 rows x 4096]:
  - two DVE adds pool the free dim (d4 pairs, then d3 pairs)
  - four accumulating fp32 matmuls with a constant pooling matrix pool the
    partition dim (d1,d2 pairs), placing results in the right PSUM row band
  - ScalarE copies PSUM->SBUF, HWDGE DMA writes the contiguous 128-row output band
The 1/16 average scale is folded into the pooling matrix.
"""

import sys

import numpy as np

if "/opt/trn_rl_repo" not in sys.path:
    sys.path.insert(0, "/opt/trn_rl_repo")

import concourse.bacc as bacc
import concourse.bass as bass
import concourse.tile as tile
from concourse import mybir
from concourse.bass_utils import run_bass_kernel_spmd

N_CORES = 8
SLICES_PER_CORE = 4  # 32 (b,c) slices / 8 cores
ROWS = SLICES_PER_CORE * 1024  # 4096
N_LOADS = ROWS // 512  # 8 x [128,4096] tiles
F32 = mybir.dt.float32


def _build_pm() -> np.ndarray:
    # pm[r, q, m] = 1/16 iff matmul q maps input row r=32a+d2 (a=d1 mod 4 block)
    # to psum row m = 32q + 16*(a//2) + d2//2
    pm = np.zeros((128, 4, 128), np.float32)
    for a in range(4):
        for d2 in range(32):
            r = 32 * a + d2
            for q in range(4):
                m = 32 * q + 16 * (a // 2) + d2 // 2
                pm[r, q, m] = 1.0 / 16.0
    return np.ascontiguousarray(pm.reshape(128, 512))


def build_nc() -> bass.Bass:
    # Bacc (not raw Bass): its compile() splits multi-sem sync waits into
    # event-semaphore instructions (TRN2 allows one wait per instruction)
    # and moves matmul waits onto ldweights.
    nc = bacc.Bacc()
    x = nc.dram_tensor("x", [ROWS, 1024], F32, kind="ExternalInput")
    pm = nc.dram_tensor("pm", [128, 512], F32, kind="ExternalInput")
    y = nc.dram_tensor("y", [ROWS // 4, 256], F32, kind="ExternalOutput")

    with tile.TileContext(nc) as tc:
        with (
            tc.tile_pool(name="pmp", bufs=1) as pmp,
            # bufs = N_LOADS everywhere: no tile-slot reuse, so DMA and
            # Matmult instructions carry at most one sync wait each (their
            # ISA limit). The whole 16 MiB input lives in SBUF at once.
            tc.tile_pool(name="inp", bufs=N_LOADS) as inp,
            tc.tile_pool(name="mid1", bufs=2) as mid1p,
            tc.tile_pool(name="mid2", bufs=2) as mid2p,
            tc.tile_pool(name="psp", bufs=N_LOADS, space=bass.MemorySpace.PSUM) as psp,
            tc.tile_pool(name="outp", bufs=N_LOADS) as outp,
        ):
            pm_t = pmp.tile([128, 512], F32)
            nc.sync.dma_start(pm_t[:], pm[:])

            for l in range(N_LOADS):
                t = inp.tile([128, 4096], F32)
                src = x[512 * l : 512 * (l + 1), :].rearrange(
                    "(q p) c -> p q c", p=128
                )
                nc.sync.dma_start(t[:].rearrange("p (q c) -> p q c", q=4), src)

                # pool d4 pairs: [128, 4q, 32d3, 16o4, 2e4] -> [128, 4, 32, 16]
                m1 = mid1p.tile([128, 2048], F32)
                v = t[:].rearrange(
                    "p (q d3 o4 e4) -> p q d3 o4 e4", q=4, d3=32, o4=16, e4=2
                )
                m1v = m1[:].rearrange("p (q d3 o4) -> p q d3 o4", q=4, d3=32)
                nc.vector.tensor_add(m1v, v[:, :, :, :, 0], v[:, :, :, :, 1])

                # pool d3 pairs: [128, 4q, 16o3, 2e3, 16o4] -> [128, 4, 16, 16]
                m2 = mid2p.tile([128, 1024], F32)
                w = m1[:].rearrange(
                    "p (q o3 e3 o4) -> p q o3 e3 o4", q=4, o3=16, e3=2
                )
                m2v = m2[:].rearrange("p (q o3 o4) -> p q o3 o4", q=4, o3=16)
                nc.vector.tensor_add(m2v, w[:, :, :, 0, :], w[:, :, :, 1, :])

                # pool (d1,d2) pairs across partitions: 4 banded matmuls
                ps = psp.tile([128, 256], F32, tag="ps")
                if l == 0:
                    # Warmup matmul: first PE instruction, absorbs the one
                    # allowed sync wait (pm_t's DMA) so the real matmuls only
                    # wait on DVE. Targets load-0's own psum tile (overwritten
                    # by the start=True matmul below) to avoid slot-reuse sems.
                    nc.tensor.matmul(ps[:, 0:1], pm_t[:, 0:128], pm_t[:, 0:1])
                m2q = m2[:].rearrange("p (q n) -> p q n", q=4)
                for q in range(4):
                    nc.tensor.matmul(
                        ps[:],
                        pm_t[:, 128 * q : 128 * (q + 1)],
                        m2q[:, q, :],
                        start=(q == 0),
                        stop=(q == 3),
                    )

                ob = outp.tile([128, 256], F32)
                nc.scalar.copy(ob[:], ps[:])
                nc.scalar.dma_start(y[128 * l : 128 * (l + 1), :], ob[:])

    nc.compile()
    return nc


_NC_CACHE: bass.Bass | None = None


def kernel(nd_tensor: np.ndarray, _trace: bool = False):
    global _NC_CACHE
    x = np.ascontiguousarray(np.asarray(nd_tensor, dtype=np.float32)).reshape(
        32, 1024, 1024
    )
    pm = _build_pm()
    if _NC_CACHE is None:
        _NC_CACHE = build_nc()
    nc = _NC_CACHE

    in_maps = [
        {
            "x": np.ascontiguousarray(
                x[SLICES_PER_CORE * i : SLICES_PER_CORE * (i + 1)]
            ).reshape(ROWS, 1024),
            "pm": pm,
        }
        for i in range(N_CORES)
    ]
    res = run_bass_kernel_spmd(
        nc, in_maps, core_ids=list(range(N_CORES)), trace=_trace
    )
    out = np.stack([res.results[i]["y"] for i in range(N_CORES)])  # [8,1024,256]
    out = out.reshape(2, 16, 16, 16, 16, 16).astype(np.float32)
    if _trace:
        kernel.last_results = res
    return out
